# revision 35
# baseline (speedup 1.0000x reference)
"""Single-head attention with per-sample padding masks on 8 Trainium2
NeuronCores — length-aware work rebalancing.

kernel(**inputs) takes the FULL unsharded inputs and returns the FULL
[B, N, D] float32 output.

The per-sample event_lengths are known when kernel() is called, so the
device program is built (and cached) per lengths-tuple.  Valid attention
work scales as (L/N)^2 per sample; instead of one sample per core, the
512-query BLOCKS of all samples are bin-packed across the 8 cores:

  - Each core owns QB = ceil(total_blocks/8) query-block slots and a
    packed key-set of up to KT=16 key tiles (128 rows each).  A core's
    key-set concatenates the key tiles of every sample whose blocks it
    hosts; the per-(key-tile, block) mask bias (0 valid / -1e9) closes
    cross-sample and padded-key positions, so packing costs nothing.
  - Padded-query rows (i >= L) need softmax over an all-masked row =
    mean of v over ALL N rows; that is colsum(x) @ Wv.T / N + bv,
    computed EXACTLY on the host and scattered in during assembly, so
    the device mean path (colsum matmuls, mean replication, output
    blend) disappears.
  - Idle slots (capacity rounding) attend tile 0 of the core's key-set
    with an open mask so rowsum >= 1 (finite garbage, discarded).

Device numerics are unchanged from the tuned baseline (8.9e-3 rel err):
residual-compensated fp8 DoubleRow matmuls, W' = WSCALE*W staged as fp8
pairs, q'/k' quantized on-device to fp8 pairs, scores 2-term
(k8q8 + k8qr), exp via ACT with the mask riding the bias, AV 3-term
(a8v8 + a8vr + arv8) with a 2-term rowsum, out = AV * 1/rowsum.
"""

import math
import sys
from contextlib import ExitStack

import numpy as np

sys.path.insert(0, "/opt/trn_rl_repo")

import concourse.mybir as mybir  # noqa: E402
import concourse.tile as tile  # noqa: E402
from concourse import bacc  # noqa: E402

P = 128
B, N, D = 8, 2048, 512
FB = 512  # psum free-dim block (one bank) = query-block width
KT = 16  # key tiles per core (packed key-set capacity)
MASK_VAL = -1.0e9
# Weights pre-scaled into fp8 normal range.  32 (not 64): q' = WSCALE*q must
# stay below fp8 e4m3 max 240.
WSCALE = 32.0


def plan_assignment(lens, n=N):
    """Bin-pack 512-query blocks onto 8 cores.

    Returns (QB, slots) where slots[c] is a list of length QB of either
    (sample, block_idx) or None (idle), plus keysets[c]: ordered list of
    (sample, kt_count) giving the packed key-tile layout of core c.
    """
    lens = [int(l) for l in lens]
    nb_s = [max(1, math.ceil(l / FB)) for l in lens]
    kt_s = [max(1, math.ceil(l / P)) for l in lens]
    total = sum(nb_s)
    QB = max(1, math.ceil(total / 8))
    while True:
        order = sorted(range(len(lens)), key=lambda b: -kt_s[b])
        slots = [[] for _ in range(8)]
        keysets = [[] for _ in range(8)]
        keyused = [0] * 8
        ok = True
        for b in order:
            remaining = nb_s[b]
            while remaining > 0:
                best, best_cost = None, None
                for c in range(8):
                    if len(slots[c]) >= QB:
                        continue
                    add = 0 if any(s == b for s, _ in keysets[c]) else kt_s[b]
                    if keyused[c] + add > KT:
                        continue
                    space = QB - len(slots[c])
                    # best-fit: least keyset growth, then tightest slot fit
                    cost = (add, space)
                    if best is None or cost < best_cost:
                        best, best_cost = c, cost
                if best is None:
                    ok = False
                    break
                c = best
                if not any(s == b for s, _ in keysets[c]):
                    keysets[c].append((b, kt_s[b]))
                    keyused[c] += kt_s[b]
                take = min(remaining, QB - len(slots[c]))
                start = nb_s[b] - remaining
                for j in range(start, start + take):
                    slots[c].append((b, j))
                remaining -= take
            if not ok:
                break
        if ok:
            for c in range(8):
                while len(slots[c]) < QB:
                    slots[c].append(None)
            return QB, slots, keysets
        QB += 1


def build_attention_nc(qb, n=N, d=D, debug=False):
    """Build the one-core Bass program for QB query blocks x KT key tiles."""
    f32 = mybir.dt.float32
    bf16 = mybir.dt.bfloat16
    fp8 = mybir.dt.float8e4
    DR = mybir.MatmulPerfMode.DoubleRow
    ec_n = d // P  # embedding chunks (contraction over E and D)
    nq = qb * FB  # query columns on this core
    nk = KT * P  # packed key rows on this core
    s = 1.0 / math.sqrt(d)

    nc = bacc.Bacc(None, target_bir_lowering=False, debug=debug)

    # inputs staged host-side in [P, chunk, cols] layout: contiguous per
    # partition
    xq8_d = nc.declare_dram_parameter("xq8", [P, ec_n, nq], fp8, isOutput=False)
    xqr_d = nc.declare_dram_parameter("xqr", [P, ec_n, nq], fp8, isOutput=False)
    xk8_d = nc.declare_dram_parameter("xk8", [P, ec_n, nk], fp8, isOutput=False)
    xkr_d = nc.declare_dram_parameter("xkr", [P, ec_n, nk], fp8, isOutput=False)
    w8_ds, wr_ds = {}, {}
    for wn in ("wq", "wk", "wv"):
        w8_ds[wn] = nc.declare_dram_parameter(
            wn + "8", [P, ec_n, d], fp8, isOutput=False
        )
        wr_ds[wn] = nc.declare_dram_parameter(
            wn + "r", [P, ec_n, d], fp8, isOutput=False
        )
    bq_d = nc.declare_dram_parameter("bq", [P, ec_n], f32, isOutput=False)
    bk_d = nc.declare_dram_parameter("bk", [P, ec_n], f32, isOutput=False)
    maskb_d = nc.declare_dram_parameter("maskb", [P, KT, qb], f32, isOutput=False)
    out_d = nc.declare_dram_parameter("out", [nq, d], f32, isOutput=True)

    Ident = mybir.ActivationFunctionType.Identity
    Exp = mybir.ActivationFunctionType.Exp
    Add = mybir.AluOpType.add
    Mult = mybir.AluOpType.mult
    Sub = mybir.AluOpType.subtract

    with tile.TileContext(nc) as tc, ExitStack() as ctx:
        const = ctx.enter_context(tc.tile_pool(name="const", bufs=1))
        big = ctx.enter_context(tc.tile_pool(name="big", bufs=1))
        work = ctx.enter_context(tc.tile_pool(name="work", bufs=8))
        small = ctx.enter_context(tc.tile_pool(name="small", bufs=4))
        psum_s = ctx.enter_context(tc.tile_pool(name="psum_s", bufs=4, space="PSUM"))
        psum_av = ctx.enter_context(tc.tile_pool(name="psum_av", bufs=3, space="PSUM"))
        psum_rs = ctx.enter_context(tc.tile_pool(name="psum_rs", bufs=1, space="PSUM"))

        # ---- constants / parameters into SBUF ----
        # warmup operands first: their memsets gate the p-state warmup
        # matmuls, everything else can land later
        ones_row = const.tile([1, P], bf16)
        nc.vector.memset(ones_row, 1.0)
        warm = const.tile([1, FB], bf16)
        nc.vector.memset(warm, 1.0)
        bq_sb = const.tile([P, ec_n], f32)
        bk_sb = const.tile([P, ec_n], f32)
        maskb_sb = const.tile([P, KT, qb], f32)

        xq8_sb = big.tile([P, ec_n, nq], fp8)
        xqr_sb = big.tile([P, ec_n, nq], fp8)
        xk8_sb = big.tile([P, ec_n, nk], fp8)
        xkr_sb = big.tile([P, ec_n, nk], fp8)
        w8_sb = {wn: big.tile([P, ec_n, d], fp8, name=wn + "8_sb") for wn in ("wq", "wk", "wv")}
        wr_sb = {wn: big.tile([P, ec_n, d], fp8, name=wn + "r_sb") for wn in ("wq", "wk", "wv")}
        # coalesced contiguous DMAs spread over the 3 DMA-capable queues;
        # K-side first (K-proj opens the pipeline), then Q-side, then V.
        # wk split in ec-pair halves: the opening K-proj DoubleRow only
        # needs ecp 0..1 of w8/wr, so the PE starts ~0.8us earlier
        nc.scalar.dma_start(out=w8_sb["wk"][:, 0:2, :], in_=w8_ds["wk"][:, 0:2, :])
        nc.scalar.dma_start(out=wr_sb["wk"][:, 0:2, :], in_=wr_ds["wk"][:, 0:2, :])
        nc.scalar.dma_start(out=w8_sb["wk"][:, 2:4, :], in_=w8_ds["wk"][:, 2:4, :])
        nc.scalar.dma_start(out=wr_sb["wk"][:, 2:4, :], in_=wr_ds["wk"][:, 2:4, :])
        nkb = nk // FB
        for ib in range(nkb):
            if ib == 0:
                # first block split in dc halves so the opening DoubleRow
                # pair (dc 0..1) lands earlier
                for h in range(2):
                    nc.sync.dma_start(
                        out=xk8_sb[:, 2 * h : 2 * h + 2, 0:FB],
                        in_=xk8_d[:, 2 * h : 2 * h + 2, 0:FB],
                    )
                    nc.gpsimd.dma_start(
                        out=xkr_sb[:, 2 * h : 2 * h + 2, 0:FB],
                        in_=xkr_d[:, 2 * h : 2 * h + 2, 0:FB],
                    )
                nc.gpsimd.dma_start(out=bq_sb, in_=bq_d[:, :])
                nc.gpsimd.dma_start(out=bk_sb, in_=bk_d[:, :])

            else:
                nc.sync.dma_start(
                    out=xk8_sb[:, :, ib * FB : (ib + 1) * FB],
                    in_=xk8_d[:, :, ib * FB : (ib + 1) * FB],
                )
                nc.gpsimd.dma_start(
                    out=xkr_sb[:, :, ib * FB : (ib + 1) * FB],
                    in_=xkr_d[:, :, ib * FB : (ib + 1) * FB],
                )
            if ib == 0:
                nc.sync.dma_start(out=w8_sb["wq"], in_=w8_ds["wq"][:, :, :])
                nc.sync.dma_start(out=wr_sb["wq"], in_=wr_ds["wq"][:, :, :])
                nc.sync.dma_start(out=xq8_sb[:, :, 0:FB], in_=xq8_d[:, :, 0:FB])
                nc.gpsimd.dma_start(out=xqr_sb[:, :, 0:FB], in_=xqr_d[:, :, 0:FB])
            elif ib == 1:
                nc.scalar.dma_start(out=w8_sb["wv"], in_=w8_ds["wv"][:, :, :])
                nc.scalar.dma_start(out=wr_sb["wv"], in_=wr_ds["wv"][:, :, :])
                nc.gpsimd.dma_start(out=maskb_sb, in_=maskb_d[:, :, :])
        for ib in range(1, qb):
            nc.sync.dma_start(
                out=xq8_sb[:, :, ib * FB : (ib + 1) * FB],
                in_=xq8_d[:, :, ib * FB : (ib + 1) * FB],
            )
            nc.gpsimd.dma_start(
                out=xqr_sb[:, :, ib * FB : (ib + 1) * FB],
                in_=xqr_d[:, :, ib * FB : (ib + 1) * FB],
            )

        q8_sb = big.tile([P, ec_n, nq], fp8)
        qr_sb = big.tile([P, ec_n, nq], fp8)
        k8_sb = big.tile([P, ec_n, nk], fp8)
        kr_sb = big.tile([P, ec_n, nk], fp8)
        v8_sb = big.tile([P, KT, d], fp8)
        vr_sb = big.tile([P, KT, d], fp8)
        a8_sb = big.tile([P, KT, nq], fp8)
        ar_sb = big.tile([P, KT, nq], fp8)
        # p-state warm-up: keep the PE busy during the initial DMA window so
        # real work starts at full clock
        for _ in range(10):
            pd = psum_rs.tile([P, FB], f32, tag="prs")
            nc.tensor.matmul(pd, lhsT=ones_row, rhs=warm, start=True, stop=True)
        # rowsum is taken against WSCALE (not 1) so 1/rowsum' also cancels
        # the WSCALE carried by v' in the AV numerator: num·W / (den·W)
        ones2_col = const.tile([P, 2, 1], fp8)
        nc.vector.memset(ones2_col, WSCALE)
        zero_d = const.tile([P, d], f32)
        nc.vector.memset(zero_d, 0.0)
        zero_1 = const.tile([P, 1], f32)
        nc.vector.memset(zero_1, 0.0)

        def mm3(ps, a8, ar, b8, br, asl, bsl, drop_ar=False):
            """psum += a@b as a8@b8 + a8@br + ar@b8 (fp8 DoubleRow terms).
            drop_ar: omit ar@b8 (scores only; measured +8.6e-3 in quadrature,
            total 8.9e-3 vs the 2e-2 gate)."""
            terms = [(a8, b8), (a8, br)]
            if not drop_ar:
                terms.append((ar, b8))
            nmm = len(terms) * (ec_n // 2)
            i = 0
            for ta, tb in terms:
                for ecp in range(0, ec_n, 2):
                    nc.tensor.matmul(
                        ps,
                        lhsT=asl(ta, ecp),
                        rhs=bsl(tb, ecp),
                        start=(i == 0),
                        stop=(i == nmm - 1),
                        perf_mode=DR,
                    )
                    i += 1

        # ---- projections; W-residuals are NOT droppable (uniform weights
        # quantize 1.6x worse; measured 1.7e-2 alone) ----
        def emit_proj_ec(wn, b_sb, o8, orr, x8, xr, ib, ec, fill=False):
            if fill:
                # filler inside scores(0): stay off psum_s (the scores
                # rotation) -- use the pools that are idle in that phase
                if ec % 2 == 0:
                    ps = psum_rs.tile([P, FB], f32, tag="prs", name="ps_qk")
                else:
                    ps = psum_av.tile([P, FB], f32, tag="pav", name="ps_qk")
            elif ec % 2 == 0:
                ps = psum_s.tile([P, FB], f32, tag="ps", name="ps_qk")
            else:
                ps = psum_av.tile([P, FB], f32, tag="pav", name="ps_qk")
            mm3(
                ps,
                w8_sb[wn],
                wr_sb[wn],
                x8,
                xr,
                lambda t, ecp, ec=ec: t[:, ecp : ecp + 2, ec * P : (ec + 1) * P],
                lambda t, ecp, ib=ib: t[:, ecp : ecp + 2, ib * FB : (ib + 1) * FB],
            )
            sl = (slice(None), ec, slice(ib * FB, (ib + 1) * FB))
            nc.scalar.activation(
                o8[sl], ps, Ident, bias=b_sb[:, ec : ec + 1], scale=1.0
            )
            nc.vector.scalar_tensor_tensor(
                out=orr[sl],
                in0=ps,
                scalar=b_sb[:, ec : ec + 1],
                in1=o8[sl],
                op0=Add,
                op1=Sub,
            )

        def emit_proj_unit(wn, b_sb, o8, orr, x8, xr, ib):
            for ec in range(ec_n):
                emit_proj_ec(wn, b_sb, o8, orr, x8, xr, ib, ec)

        def emit_vproj(jt, fill=False):
            if fill:
                ps = psum_rs.tile([P, d], f32, tag="prs")
            else:
                ps = psum_s.tile([P, d], f32, tag="ps")
            mm3(
                ps,
                xk8_sb,
                xkr_sb,
                w8_sb["wv"],
                wr_sb["wv"],
                lambda t, ecp, jt=jt: t[:, ecp : ecp + 2, jt * P : (jt + 1) * P],
                lambda t, ecp: t[:, ecp : ecp + 2, :],
            )
            # v8 straight from PSUM (ACT), vr = psum - v8 (DVE) -- the bf16
            # intermediate (and its Pool copy) served only the device mean
            # path, which now lives on the host
            nc.scalar.activation(v8_sb[:, jt, :], ps, Ident, bias=0.0, scale=1.0)
            nc.vector.scalar_tensor_tensor(
                out=vr_sb[:, jt, :],
                in0=ps,
                scalar=zero_1,
                in1=v8_sb[:, jt, :],
                op0=Add,
                op1=Sub,
            )

        # ---- attention ----
        def emit_scores_unit(ib, jt):
            ps = psum_s.tile([P, FB], f32, tag="ps")
            mm3(
                ps,
                k8_sb,
                kr_sb,
                q8_sb,
                qr_sb,
                lambda t, ecp, jt=jt: t[:, ecp : ecp + 2, jt * P : (jt + 1) * P],
                lambda t, ecp, ib=ib: t[:, ecp : ecp + 2, ib * FB : (ib + 1) * FB],
                drop_ar=True,
            )
            ab = work.tile([P, FB], bf16)
            nc.scalar.activation(
                ab,
                ps,
                Exp,
                bias=maskb_sb[:, jt, ib : ib + 1],
                scale=s / (WSCALE * WSCALE),
            )
            sl = (slice(None), jt, slice(ib * FB, (ib + 1) * FB))
            nc.gpsimd.tensor_copy(a8_sb[sl], ab)
            nc.vector.tensor_tensor(out=ar_sb[sl], in0=ab, in1=a8_sb[sl], op=Sub)

        def emit_scores(ib, av_of=None):
            # av_of: interleave the 4 AV chunks of that (older, data-ready)
            # block between this block's scores units
            for jt in range(KT):
                if av_of is not None and jt >= 4 and (jt - 4) % 4 == 0:
                    emit_av_chunk(av_of * 4 + (jt - 4) // 4)
                emit_scores_unit(ib, jt)
            if av_of is not None:
                emit_av_chunk(av_of * 4 + 3)

        def emit_rowsum(it):
            prs = psum_rs.tile([P, 1], f32, tag="prs")
            i = 0
            for ta in (a8_sb, ar_sb):
                for jtp in range(0, KT, 2):
                    nc.tensor.matmul(
                        prs,
                        lhsT=ta[:, jtp : jtp + 2, it * P : (it + 1) * P],
                        rhs=ones2_col,
                        start=(i == 0),
                        stop=(i == 2 * (KT // 2) - 1),
                        perf_mode=DR,
                    )
                    i += 1
            rinv = small.tile([P, 1], f32)
            nc.vector.reciprocal(rinv, prs)
            return rinv

        av_terms = [(a8_sb, v8_sb), (a8_sb, vr_sb), (ar_sb, v8_sb)]

        def emit_av_half(pav, it, hd, last):
            i = 0
            nmm = len(av_terms) * (KT // 2)
            for ta, tv in av_terms:
                for jtp in range(0, KT, 2):
                    nc.tensor.matmul(
                        pav[:, hd] if hd is not None else pav,
                        lhsT=ta[:, jtp : jtp + 2, it * P : (it + 1) * P],
                        rhs=tv[:, jtp : jtp + 2, hd if hd is not None else slice(None)],
                        start=(i == 0),
                        stop=(i == nmm - 1),
                        perf_mode=DR,
                    )
                    i += 1

        def emit_av_chunk(it):
            pav = psum_av.tile([P, d], f32, tag="pav")
            emit_av_half(pav, it, None, False)
            rinv = emit_rowsum(it)
            if it == qb * 4 - 1:
                # last chunk: halve the epilogue so the final output DMA
                # (on the kernel's drain path) starts earlier
                outt = work.tile([P, d], f32)
                for h, q in ((0, nc.sync), (1, nc.scalar)):
                    hd = slice(h * (d // 2), (h + 1) * (d // 2))
                    nc.vector.scalar_tensor_tensor(
                        out=outt[:, hd],
                        in0=pav[:, hd],
                        scalar=rinv,
                        in1=zero_d[:, hd],
                        op0=Mult,
                        op1=Add,
                    )
                    q.dma_start(
                        out=out_d[it * P : (it + 1) * P, hd], in_=outt[:, hd]
                    )
                return
            outt = work.tile([P, d], f32)
            nc.vector.scalar_tensor_tensor(
                out=outt,
                in0=pav,
                scalar=rinv,
                in1=zero_d,
                op0=Mult,
                op1=Add,
            )
            nc.sync.dma_start(out=out_d[it * P : (it + 1) * P, :], in_=outt)

        # software-pipelined emission:
        #  - K projection first (scores(0) needs it), Q and V interleaved;
        #  - scores(0) slotted in as soon as k-columns + q-block 0 exist;
        #  - AV chunks of block ib interleave into scores(ib+1).
        nkb = nk // FB
        for ib in range(nkb):
            emit_proj_unit("wk", bk_sb, k8_sb, kr_sb, xk8_sb, xkr_sb, ib)
            if ib < min(qb, 2):
                emit_proj_unit("wq", bq_sb, q8_sb, qr_sb, xq8_sb, xqr_sb, ib)
            for jt in range(ib * 4, ib * 4 + 4):
                if jt < 12:
                    emit_vproj(jt)
        # scores(0) has no AV filler (nothing ready yet) and the 2-term
        # scores PE outpaces the exp -> consumer chain; V(12-15) and the
        # deferred Q-proj blocks (>=2) slot in as PE filler.
        qfill_ec = {}
        for i, ib2 in enumerate(range(2, qb)):
            for ec in range(ec_n):
                qfill_ec[4 + 3 * (i * ec_n + ec)] = (ib2, ec)
        for jt in range(KT):
            if jt in (5, 8, 11, 14):
                emit_vproj((jt - 5) // 3 + 12, fill=True)
            if jt in qfill_ec:
                ib2, ec = qfill_ec[jt]
                emit_proj_ec(
                    "wq", bq_sb, q8_sb, qr_sb, xq8_sb, xqr_sb, ib2, ec, fill=True
                )
            emit_scores_unit(0, jt)
        for ib in range(1, qb):
            emit_scores(ib, av_of=ib - 1)
        for it in range((qb - 1) * 4, qb * 4):
            emit_av_chunk(it)

    nc.compile()
    return nc


def _fp8_pair(a, npdt):
    a = np.asarray(a, np.float32)
    a8 = a.astype(npdt)
    ar = (a - a8.astype(np.float32)).astype(npdt)
    return a8, ar


def _pcn(a):
    """[(c p), n] -> [p, c, n] partition-major layout."""
    a = np.asarray(a)
    cn, n = a.shape
    return np.ascontiguousarray(a.reshape(cn // P, P, n).transpose(1, 0, 2))


def make_in_maps(x, event_lengths, Wq, bq, Wk, bk, Wv, bv):
    """Host-side planning + marshaling. Returns (in_maps, plan)."""
    npdt = mybir.dt.np(mybir.dt.float8e4)
    x = np.asarray(x, dtype=np.float32)
    lens = np.asarray(event_lengths).astype(np.int64)
    qb, slots, keysets = plan_assignment(lens)
    nq = qb * FB
    nk = KT * P
    ws = {}
    for wn, W in (("wq", Wq), ("wk", Wk), ("wv", Wv)):
        wT = np.ascontiguousarray(np.asarray(W, np.float32).T) * WSCALE
        w8, wr = _fp8_pair(wT, npdt)
        ws[wn + "8"], ws[wn + "r"] = _pcn(w8), _pcn(wr)
    bq_m = np.ascontiguousarray(
        (np.asarray(bq, np.float32) * np.float32(WSCALE)).reshape(D // P, P).T
    )
    bk_m = np.ascontiguousarray(
        (np.asarray(bk, np.float32) * np.float32(WSCALE)).reshape(D // P, P).T
    )
    xT = {b: np.ascontiguousarray(x[b].T) for b in range(x.shape[0])}
    in_maps = []
    for c in range(8):
        # packed key-side x: concat keyset samples' first kt*128 seq cols
        xk = np.zeros((D, nk), np.float32)
        ktpos = {}  # sample -> starting key tile
        pos = 0
        for b, kt in keysets[c]:
            ktpos[b] = pos
            xk[:, pos * P : pos * P + kt * P] = xT[b][:, : kt * P]
            pos += kt
        # query-side x: per slot, that sample's block columns
        xq = np.zeros((D, nq), np.float32)
        maskb = np.full((P, KT, qb), MASK_VAL, np.float32)
        for j, slot in enumerate(slots[c]):
            if slot is None:
                maskb[:, 0, j] = 0.0  # keep rowsum >= 1; output discarded
                continue
            b, blk = slot
            xq[:, j * FB : (j + 1) * FB] = xT[b][:, blk * FB : (blk + 1) * FB]
            base = ktpos[b]
            ktn = dict(keysets[c])[b]
            L = int(lens[b])
            for t in range(ktn):
                valid = (t * P + np.arange(P)) < L
                maskb[:, base + t, j] = np.where(valid, 0.0, MASK_VAL)
        xk8, xkr = _fp8_pair(xk, npdt)
        xq8, xqr = _fp8_pair(xq, npdt)
        in_maps.append(
            {
                "xq8": _pcn(xq8),
                "xqr": _pcn(xqr),
                "xk8": _pcn(xk8),
                "xkr": _pcn(xkr),
                **ws,
                "bq": bq_m,
                "bk": bk_m,
                "maskb": maskb,
            }
        )
    return in_maps, (qb, slots)


_NC_CACHE = {}


def kernel(x, event_lengths, Wq, bq, Wk, bk, Wv, bv):
    from concourse.bass_utils import run_bass_kernel_spmd

    x = np.asarray(x, np.float32)
    lens = np.asarray(event_lengths).astype(np.int64)
    in_maps, (qb, slots) = make_in_maps(x, lens, Wq, bq, Wk, bk, Wv, bv)
    if qb not in _NC_CACHE:
        _NC_CACHE[qb] = build_attention_nc(qb)
    nc = _NC_CACHE[qb]
    res = run_bass_kernel_spmd(nc, in_maps, core_ids=list(range(8)))
    # host assembly: scatter core blocks, then fill padded-query rows with
    # the exact mean of v over all N rows (softmax over an all-masked row).
    Wv32 = np.asarray(Wv, np.float32)
    bv32 = np.asarray(bv, np.float32)
    mean_v = (x.sum(axis=1) @ Wv32.T) / np.float32(N) + bv32  # [B, D]
    out = np.empty((B, N, D), np.float32)
    for c in range(8):
        co = np.asarray(res.results[c]["out"], np.float32)
        for j, slot in enumerate(slots[c]):
            if slot is None:
                continue
            b, blk = slot
            out[b, blk * FB : (blk + 1) * FB, :] = co[j * FB : (j + 1) * FB, :]
    for b in range(B):
        L = int(lens[b])
        if L < N:
            out[b, L:, :] = mean_v[b][None, :]
    return out


# revision 36
# speedup vs baseline: 1.1315x; 1.1315x over previous
"""Single-head attention with per-sample padding masks on 8 Trainium2
NeuronCores — length-aware work rebalancing.

kernel(**inputs) takes the FULL unsharded inputs and returns the FULL
[B, N, D] float32 output.

The per-sample event_lengths are known when kernel() is called, so the
device program is built (and cached) per lengths-tuple.  Valid attention
work scales as (L/N)^2 per sample; instead of one sample per core, the
512-query BLOCKS of all samples are bin-packed across the 8 cores:

  - Each core owns QB = ceil(total_blocks/8) query-block slots and a
    packed key-set of up to KT=16 key tiles (128 rows each).  A core's
    key-set concatenates the key tiles of every sample whose blocks it
    hosts; the per-(key-tile, block) mask bias (0 valid / -1e9) closes
    cross-sample and padded-key positions, so packing costs nothing.
  - Padded-query rows (i >= L) need softmax over an all-masked row =
    mean of v over ALL N rows; that is colsum(x) @ Wv.T / N + bv,
    computed EXACTLY on the host and scattered in during assembly, so
    the device mean path (colsum matmuls, mean replication, output
    blend) disappears.
  - Idle slots (capacity rounding) attend tile 0 of the core's key-set
    with an open mask so rowsum >= 1 (finite garbage, discarded).

Device numerics are unchanged from the tuned baseline (8.9e-3 rel err):
residual-compensated fp8 DoubleRow matmuls, W' = WSCALE*W staged as fp8
pairs, q'/k' quantized on-device to fp8 pairs, scores 2-term
(k8q8 + k8qr), exp via ACT with the mask riding the bias, AV 3-term
(a8v8 + a8vr + arv8) with a 2-term rowsum, out = AV * 1/rowsum.
"""

import math
import sys
from contextlib import ExitStack

import numpy as np

sys.path.insert(0, "/opt/trn_rl_repo")

import concourse.mybir as mybir  # noqa: E402
import concourse.tile as tile  # noqa: E402
from concourse import bacc  # noqa: E402

P = 128
B, N, D = 8, 2048, 512
FB = 512  # psum free-dim block (one bank) = query-block width
KT = 16  # key tiles per core (packed key-set capacity)
MASK_VAL = -1.0e9
# Weights pre-scaled into fp8 normal range.  32 (not 64): q' = WSCALE*q must
# stay below fp8 e4m3 max 240.
WSCALE = 32.0


def plan_assignment(lens, n=N):
    """Bin-pack 512-query blocks onto 8 cores.

    Returns (QB, slots) where slots[c] is a list of length QB of either
    (sample, block_idx) or None (idle), plus keysets[c]: ordered list of
    (sample, kt_count) giving the packed key-tile layout of core c.
    """
    lens = [int(l) for l in lens]
    nb_s = [max(1, math.ceil(l / FB)) for l in lens]
    kt_s = [max(1, math.ceil(l / P)) for l in lens]
    total = sum(nb_s)
    QB = max(1, math.ceil(total / 8))
    while True:
        order = sorted(range(len(lens)), key=lambda b: -kt_s[b])
        slots = [[] for _ in range(8)]
        keysets = [[] for _ in range(8)]
        keyused = [0] * 8
        ok = True
        for b in order:
            remaining = nb_s[b]
            while remaining > 0:
                best, best_cost = None, None
                for c in range(8):
                    if len(slots[c]) >= QB:
                        continue
                    add = 0 if any(s == b for s, _ in keysets[c]) else kt_s[b]
                    if keyused[c] + add > KT:
                        continue
                    space = QB - len(slots[c])
                    # best-fit: least keyset growth, then tightest slot fit
                    cost = (add, space)
                    if best is None or cost < best_cost:
                        best, best_cost = c, cost
                if best is None:
                    ok = False
                    break
                c = best
                if not any(s == b for s, _ in keysets[c]):
                    keysets[c].append((b, kt_s[b]))
                    keyused[c] += kt_s[b]
                take = min(remaining, QB - len(slots[c]))
                start = nb_s[b] - remaining
                for j in range(start, start + take):
                    slots[c].append((b, j))
                remaining -= take
            if not ok:
                break
        if ok:
            for c in range(8):
                while len(slots[c]) < QB:
                    slots[c].append(None)
            return QB, slots, keysets
        QB += 1


def build_attention_nc(qb, n=N, d=D, debug=False):
    """Build the one-core Bass program for QB query blocks x KT key tiles."""
    f32 = mybir.dt.float32
    bf16 = mybir.dt.bfloat16
    fp8 = mybir.dt.float8e4
    DR = mybir.MatmulPerfMode.DoubleRow
    ec_n = d // P  # embedding chunks (contraction over E and D)
    nq = qb * FB  # query columns on this core
    nk = KT * P  # packed key rows on this core
    s = 1.0 / math.sqrt(d)

    nc = bacc.Bacc(None, target_bir_lowering=False, debug=debug)

    # inputs staged host-side in [P, chunk, cols] layout: contiguous per
    # partition
    xq8_d = nc.declare_dram_parameter("xq8", [P, ec_n, nq], fp8, isOutput=False)
    xqr_d = nc.declare_dram_parameter("xqr", [P, ec_n, nq], fp8, isOutput=False)
    xk8_d = nc.declare_dram_parameter("xk8", [P, ec_n, nk], fp8, isOutput=False)
    xkr_d = nc.declare_dram_parameter("xkr", [P, ec_n, nk], fp8, isOutput=False)
    w8_ds, wr_ds = {}, {}
    for wn in ("wq", "wk", "wv"):
        w8_ds[wn] = nc.declare_dram_parameter(
            wn + "8", [P, ec_n, d], fp8, isOutput=False
        )
        wr_ds[wn] = nc.declare_dram_parameter(
            wn + "r", [P, ec_n, d], fp8, isOutput=False
        )
    bq_d = nc.declare_dram_parameter("bq", [P, ec_n], f32, isOutput=False)
    bk_d = nc.declare_dram_parameter("bk", [P, ec_n], f32, isOutput=False)
    maskb_d = nc.declare_dram_parameter("maskb", [P, KT, qb], f32, isOutput=False)
    out_d = nc.declare_dram_parameter("out", [nq, d], f32, isOutput=True)

    Ident = mybir.ActivationFunctionType.Identity
    Exp = mybir.ActivationFunctionType.Exp
    Add = mybir.AluOpType.add
    Mult = mybir.AluOpType.mult
    Sub = mybir.AluOpType.subtract

    with tile.TileContext(nc) as tc, ExitStack() as ctx:
        const = ctx.enter_context(tc.tile_pool(name="const", bufs=1))
        big = ctx.enter_context(tc.tile_pool(name="big", bufs=1))
        work = ctx.enter_context(tc.tile_pool(name="work", bufs=8))
        small = ctx.enter_context(tc.tile_pool(name="small", bufs=4))
        psum_s = ctx.enter_context(tc.tile_pool(name="psum_s", bufs=4, space="PSUM"))
        psum_av = ctx.enter_context(tc.tile_pool(name="psum_av", bufs=3, space="PSUM"))
        psum_rs = ctx.enter_context(tc.tile_pool(name="psum_rs", bufs=1, space="PSUM"))

        # ---- constants / parameters into SBUF ----
        # warmup operands first: their memsets gate the p-state warmup
        # matmuls, everything else can land later
        ones_row = const.tile([1, P], bf16)
        nc.vector.memset(ones_row, 1.0)
        warm = const.tile([1, FB], bf16)
        nc.vector.memset(warm, 1.0)
        bq_sb = const.tile([P, ec_n], f32)
        bk_sb = const.tile([P, ec_n], f32)
        maskb_sb = const.tile([P, KT, qb], f32)

        xq8_sb = big.tile([P, ec_n, nq], fp8)
        xqr_sb = big.tile([P, ec_n, nq], fp8)
        xk8_sb = big.tile([P, ec_n, nk], fp8)
        xkr_sb = big.tile([P, ec_n, nk], fp8)
        w8_sb = {wn: big.tile([P, ec_n, d], fp8, name=wn + "8_sb") for wn in ("wq", "wk", "wv")}
        wr_sb = {wn: big.tile([P, ec_n, d], fp8, name=wn + "r_sb") for wn in ("wq", "wk", "wv")}
        # coalesced contiguous DMAs spread over the 3 DMA-capable queues;
        # K-side first (K-proj opens the pipeline), then Q-side, then V.
        # wk split in ec-pair halves: the opening K-proj DoubleRow only
        # needs ecp 0..1 of w8/wr, so the PE starts ~0.8us earlier
        nc.scalar.dma_start(out=w8_sb["wk"][:, 0:2, :], in_=w8_ds["wk"][:, 0:2, :])
        nc.scalar.dma_start(out=wr_sb["wk"][:, 0:2, :], in_=wr_ds["wk"][:, 0:2, :])
        nc.scalar.dma_start(out=w8_sb["wk"][:, 2:4, :], in_=w8_ds["wk"][:, 2:4, :])
        nc.scalar.dma_start(out=wr_sb["wk"][:, 2:4, :], in_=wr_ds["wk"][:, 2:4, :])
        nkb = nk // FB
        for ib in range(nkb):
            if ib == 0:
                # first block split in dc halves so the opening DoubleRow
                # pair (dc 0..1) lands earlier
                for h in range(2):
                    nc.sync.dma_start(
                        out=xk8_sb[:, 2 * h : 2 * h + 2, 0:FB],
                        in_=xk8_d[:, 2 * h : 2 * h + 2, 0:FB],
                    )
                    nc.gpsimd.dma_start(
                        out=xkr_sb[:, 2 * h : 2 * h + 2, 0:FB],
                        in_=xkr_d[:, 2 * h : 2 * h + 2, 0:FB],
                    )
                nc.gpsimd.dma_start(out=bq_sb, in_=bq_d[:, :])
                nc.gpsimd.dma_start(out=bk_sb, in_=bk_d[:, :])

            else:
                nc.sync.dma_start(
                    out=xk8_sb[:, :, ib * FB : (ib + 1) * FB],
                    in_=xk8_d[:, :, ib * FB : (ib + 1) * FB],
                )
                nc.gpsimd.dma_start(
                    out=xkr_sb[:, :, ib * FB : (ib + 1) * FB],
                    in_=xkr_d[:, :, ib * FB : (ib + 1) * FB],
                )
            if ib == 0:
                nc.sync.dma_start(out=w8_sb["wq"], in_=w8_ds["wq"][:, :, :])
                nc.sync.dma_start(out=wr_sb["wq"], in_=wr_ds["wq"][:, :, :])
                nc.sync.dma_start(out=xq8_sb[:, :, 0:FB], in_=xq8_d[:, :, 0:FB])
                nc.gpsimd.dma_start(out=xqr_sb[:, :, 0:FB], in_=xqr_d[:, :, 0:FB])
            elif ib == 1:
                nc.scalar.dma_start(out=w8_sb["wv"], in_=w8_ds["wv"][:, :, :])
                nc.scalar.dma_start(out=wr_sb["wv"], in_=wr_ds["wv"][:, :, :])
                nc.gpsimd.dma_start(out=maskb_sb, in_=maskb_d[:, :, :])
        for ib in range(1, qb):
            nc.sync.dma_start(
                out=xq8_sb[:, :, ib * FB : (ib + 1) * FB],
                in_=xq8_d[:, :, ib * FB : (ib + 1) * FB],
            )
            nc.gpsimd.dma_start(
                out=xqr_sb[:, :, ib * FB : (ib + 1) * FB],
                in_=xqr_d[:, :, ib * FB : (ib + 1) * FB],
            )

        q8_sb = big.tile([P, ec_n, nq], fp8)
        qr_sb = big.tile([P, ec_n, nq], fp8)
        k8_sb = big.tile([P, ec_n, nk], fp8)
        kr_sb = big.tile([P, ec_n, nk], fp8)
        v8_sb = big.tile([P, KT, d], fp8)
        vr_sb = big.tile([P, KT, d], fp8)
        a8_sb = big.tile([P, KT, nq], fp8)
        ar_sb = big.tile([P, KT, nq], fp8)
        # p-state warm-up: keep the PE busy during the initial DMA window so
        # real work starts at full clock
        for _ in range(10):
            pd = psum_rs.tile([P, FB], f32, tag="prs")
            nc.tensor.matmul(pd, lhsT=ones_row, rhs=warm, start=True, stop=True)
        # rowsum is taken against WSCALE (not 1) so 1/rowsum' also cancels
        # the WSCALE carried by v' in the AV numerator: num·W / (den·W)
        ones2_col = const.tile([P, 2, 1], fp8)
        nc.vector.memset(ones2_col, WSCALE)
        zero_d = const.tile([P, d], f32)
        nc.vector.memset(zero_d, 0.0)
        zero_1 = const.tile([P, 1], f32)
        nc.vector.memset(zero_1, 0.0)

        def mm3(ps, a8, ar, b8, br, asl, bsl, drop_ar=False):
            """psum += a@b as a8@b8 + a8@br + ar@b8 (fp8 DoubleRow terms).
            drop_ar: omit ar@b8 (scores only; measured +8.6e-3 in quadrature,
            total 8.9e-3 vs the 2e-2 gate)."""
            terms = [(a8, b8), (a8, br)]
            if not drop_ar:
                terms.append((ar, b8))
            nmm = len(terms) * (ec_n // 2)
            i = 0
            for ta, tb in terms:
                for ecp in range(0, ec_n, 2):
                    nc.tensor.matmul(
                        ps,
                        lhsT=asl(ta, ecp),
                        rhs=bsl(tb, ecp),
                        start=(i == 0),
                        stop=(i == nmm - 1),
                        perf_mode=DR,
                    )
                    i += 1

        # ---- projections; W-residuals are NOT droppable (uniform weights
        # quantize 1.6x worse; measured 1.7e-2 alone) ----
        def emit_proj_ec(wn, b_sb, o8, orr, x8, xr, ib, ec, fill=False):
            if fill:
                # filler inside scores(0): stay off psum_s (the scores
                # rotation) -- use the pools that are idle in that phase
                if ec % 2 == 0:
                    ps = psum_rs.tile([P, FB], f32, tag="prs", name="ps_qk")
                else:
                    ps = psum_av.tile([P, FB], f32, tag="pav", name="ps_qk")
            elif ec % 2 == 0:
                ps = psum_s.tile([P, FB], f32, tag="ps", name="ps_qk")
            else:
                ps = psum_av.tile([P, FB], f32, tag="pav", name="ps_qk")
            mm3(
                ps,
                w8_sb[wn],
                wr_sb[wn],
                x8,
                xr,
                lambda t, ecp, ec=ec: t[:, ecp : ecp + 2, ec * P : (ec + 1) * P],
                lambda t, ecp, ib=ib: t[:, ecp : ecp + 2, ib * FB : (ib + 1) * FB],
            )
            sl = (slice(None), ec, slice(ib * FB, (ib + 1) * FB))
            nc.scalar.activation(
                o8[sl], ps, Ident, bias=b_sb[:, ec : ec + 1], scale=1.0
            )
            nc.vector.scalar_tensor_tensor(
                out=orr[sl],
                in0=ps,
                scalar=b_sb[:, ec : ec + 1],
                in1=o8[sl],
                op0=Add,
                op1=Sub,
            )

        def emit_proj_unit(wn, b_sb, o8, orr, x8, xr, ib):
            for ec in range(ec_n):
                emit_proj_ec(wn, b_sb, o8, orr, x8, xr, ib, ec)

        def emit_vproj(jt, fill=False):
            if fill:
                ps = psum_rs.tile([P, d], f32, tag="prs")
            else:
                ps = psum_s.tile([P, d], f32, tag="ps")
            mm3(
                ps,
                xk8_sb,
                xkr_sb,
                w8_sb["wv"],
                wr_sb["wv"],
                lambda t, ecp, jt=jt: t[:, ecp : ecp + 2, jt * P : (jt + 1) * P],
                lambda t, ecp: t[:, ecp : ecp + 2, :],
            )
            # v8 straight from PSUM (ACT), vr = psum - v8 (DVE) -- the bf16
            # intermediate (and its Pool copy) served only the device mean
            # path, which now lives on the host
            nc.scalar.activation(v8_sb[:, jt, :], ps, Ident, bias=0.0, scale=1.0)
            nc.vector.scalar_tensor_tensor(
                out=vr_sb[:, jt, :],
                in0=ps,
                scalar=zero_1,
                in1=v8_sb[:, jt, :],
                op0=Add,
                op1=Sub,
            )

        # ---- attention ----
        def emit_scores_unit(ib, jt):
            ps = psum_s.tile([P, FB], f32, tag="ps")
            mm3(
                ps,
                k8_sb,
                kr_sb,
                q8_sb,
                qr_sb,
                lambda t, ecp, jt=jt: t[:, ecp : ecp + 2, jt * P : (jt + 1) * P],
                lambda t, ecp, ib=ib: t[:, ecp : ecp + 2, ib * FB : (ib + 1) * FB],
                drop_ar=True,
            )
            ab = work.tile([P, FB], bf16)
            nc.scalar.activation(
                ab,
                ps,
                Exp,
                bias=maskb_sb[:, jt, ib : ib + 1],
                scale=s / (WSCALE * WSCALE),
            )
            sl = (slice(None), jt, slice(ib * FB, (ib + 1) * FB))
            nc.gpsimd.tensor_copy(a8_sb[sl], ab)
            nc.vector.tensor_tensor(out=ar_sb[sl], in0=ab, in1=a8_sb[sl], op=Sub)

        def emit_scores(ib, av_of=None):
            # av_of: interleave the 4 AV chunks of that (older, data-ready)
            # block between this block's scores units
            for jt in range(KT):
                if av_of is not None and jt >= 4 and (jt - 4) % 4 == 0:
                    emit_av_chunk(av_of * 4 + (jt - 4) // 4)
                emit_scores_unit(ib, jt)
            if av_of is not None:
                emit_av_chunk(av_of * 4 + 3)

        def emit_rowsum(it):
            prs = psum_rs.tile([P, 1], f32, tag="prs")
            i = 0
            for ta in (a8_sb, ar_sb):
                for jtp in range(0, KT, 2):
                    nc.tensor.matmul(
                        prs,
                        lhsT=ta[:, jtp : jtp + 2, it * P : (it + 1) * P],
                        rhs=ones2_col,
                        start=(i == 0),
                        stop=(i == 2 * (KT // 2) - 1),
                        perf_mode=DR,
                    )
                    i += 1
            rinv = small.tile([P, 1], f32)
            nc.vector.reciprocal(rinv, prs)
            return rinv

        av_terms = [(a8_sb, v8_sb), (a8_sb, vr_sb), (ar_sb, v8_sb)]

        def emit_av_half(pav, it, hd, last):
            i = 0
            nmm = len(av_terms) * (KT // 2)
            for ta, tv in av_terms:
                for jtp in range(0, KT, 2):
                    nc.tensor.matmul(
                        pav[:, hd] if hd is not None else pav,
                        lhsT=ta[:, jtp : jtp + 2, it * P : (it + 1) * P],
                        rhs=tv[:, jtp : jtp + 2, hd if hd is not None else slice(None)],
                        start=(i == 0),
                        stop=(i == nmm - 1),
                        perf_mode=DR,
                    )
                    i += 1

        def emit_av_chunk(it):
            pav = psum_av.tile([P, d], f32, tag="pav")
            if it == qb * 4 - 1:
                # last chunk: rowsum (needs only a8/ar) ahead of the AV
                # matmuls so recip completes during them, off the drain path
                rinv = emit_rowsum(it)
                emit_av_half(pav, it, None, False)
            else:
                emit_av_half(pav, it, None, False)
                rinv = emit_rowsum(it)
            if it == qb * 4 - 1:
                # last chunk: halve the epilogue so the final output DMA
                # (on the kernel's drain path) starts earlier
                outt = work.tile([P, d], f32)
                for h, q in ((0, nc.sync), (1, nc.scalar)):
                    hd = slice(h * (d // 2), (h + 1) * (d // 2))
                    nc.vector.scalar_tensor_tensor(
                        out=outt[:, hd],
                        in0=pav[:, hd],
                        scalar=rinv,
                        in1=zero_d[:, hd],
                        op0=Mult,
                        op1=Add,
                    )
                    q.dma_start(
                        out=out_d[it * P : (it + 1) * P, hd], in_=outt[:, hd]
                    )
                return
            outt = work.tile([P, d], f32)
            nc.vector.scalar_tensor_tensor(
                out=outt,
                in0=pav,
                scalar=rinv,
                in1=zero_d,
                op0=Mult,
                op1=Add,
            )
            nc.sync.dma_start(out=out_d[it * P : (it + 1) * P, :], in_=outt)

        # software-pipelined emission:
        #  - K projection first (scores(0) needs it), Q and V interleaved;
        #  - scores(0) slotted in as soon as k-columns + q-block 0 exist;
        #  - AV chunks of block ib interleave into scores(ib+1).
        nkb = nk // FB
        for ib in range(nkb):
            emit_proj_unit("wk", bk_sb, k8_sb, kr_sb, xk8_sb, xkr_sb, ib)
            if ib < min(qb, 2):
                emit_proj_unit("wq", bq_sb, q8_sb, qr_sb, xq8_sb, xqr_sb, ib)
            for jt in range(ib * 4, ib * 4 + 4):
                if jt < 12:
                    emit_vproj(jt)
        # scores(0) has no AV filler (nothing ready yet) and the 2-term
        # scores PE outpaces the exp -> consumer chain; V(12-15) and the
        # deferred Q-proj blocks (>=2) slot in as PE filler.
        qfill_ec = {}
        for i, ib2 in enumerate(range(2, qb)):
            for ec in range(ec_n):
                qfill_ec[4 + 3 * (i * ec_n + ec)] = (ib2, ec)
        for jt in range(KT):
            if jt in (5, 8, 11, 14):
                emit_vproj((jt - 5) // 3 + 12, fill=True)
            if jt in qfill_ec:
                ib2, ec = qfill_ec[jt]
                emit_proj_ec(
                    "wq", bq_sb, q8_sb, qr_sb, xq8_sb, xqr_sb, ib2, ec, fill=True
                )
            emit_scores_unit(0, jt)
        for ib in range(1, qb):
            emit_scores(ib, av_of=ib - 1)
        for it in range((qb - 1) * 4, qb * 4):
            emit_av_chunk(it)

    nc.compile()
    return nc


def _fp8_pair(a, npdt):
    a = np.asarray(a, np.float32)
    a8 = a.astype(npdt)
    ar = (a - a8.astype(np.float32)).astype(npdt)
    return a8, ar


def _pcn(a):
    """[(c p), n] -> [p, c, n] partition-major layout."""
    a = np.asarray(a)
    cn, n = a.shape
    return np.ascontiguousarray(a.reshape(cn // P, P, n).transpose(1, 0, 2))


def make_in_maps(x, event_lengths, Wq, bq, Wk, bk, Wv, bv):
    """Host-side planning + marshaling. Returns (in_maps, plan)."""
    npdt = mybir.dt.np(mybir.dt.float8e4)
    x = np.asarray(x, dtype=np.float32)
    lens = np.asarray(event_lengths).astype(np.int64)
    qb, slots, keysets = plan_assignment(lens)
    nq = qb * FB
    nk = KT * P
    ws = {}
    for wn, W in (("wq", Wq), ("wk", Wk), ("wv", Wv)):
        wT = np.ascontiguousarray(np.asarray(W, np.float32).T) * WSCALE
        w8, wr = _fp8_pair(wT, npdt)
        ws[wn + "8"], ws[wn + "r"] = _pcn(w8), _pcn(wr)
    bq_m = np.ascontiguousarray(
        (np.asarray(bq, np.float32) * np.float32(WSCALE)).reshape(D // P, P).T
    )
    bk_m = np.ascontiguousarray(
        (np.asarray(bk, np.float32) * np.float32(WSCALE)).reshape(D // P, P).T
    )
    xT = {b: np.ascontiguousarray(x[b].T) for b in range(x.shape[0])}
    in_maps = []
    for c in range(8):
        # packed key-side x: concat keyset samples' first kt*128 seq cols
        xk = np.zeros((D, nk), np.float32)
        ktpos = {}  # sample -> starting key tile
        pos = 0
        for b, kt in keysets[c]:
            ktpos[b] = pos
            xk[:, pos * P : pos * P + kt * P] = xT[b][:, : kt * P]
            pos += kt
        # query-side x: per slot, that sample's block columns
        xq = np.zeros((D, nq), np.float32)
        maskb = np.full((P, KT, qb), MASK_VAL, np.float32)
        for j, slot in enumerate(slots[c]):
            if slot is None:
                maskb[:, 0, j] = 0.0  # keep rowsum >= 1; output discarded
                continue
            b, blk = slot
            xq[:, j * FB : (j + 1) * FB] = xT[b][:, blk * FB : (blk + 1) * FB]
            base = ktpos[b]
            ktn = dict(keysets[c])[b]
            L = int(lens[b])
            for t in range(ktn):
                valid = (t * P + np.arange(P)) < L
                maskb[:, base + t, j] = np.where(valid, 0.0, MASK_VAL)
        xk8, xkr = _fp8_pair(xk, npdt)
        xq8, xqr = _fp8_pair(xq, npdt)
        in_maps.append(
            {
                "xq8": _pcn(xq8),
                "xqr": _pcn(xqr),
                "xk8": _pcn(xk8),
                "xkr": _pcn(xkr),
                **ws,
                "bq": bq_m,
                "bk": bk_m,
                "maskb": maskb,
            }
        )
    return in_maps, (qb, slots)


_NC_CACHE = {}


def kernel(x, event_lengths, Wq, bq, Wk, bk, Wv, bv):
    from concourse.bass_utils import run_bass_kernel_spmd

    x = np.asarray(x, np.float32)
    lens = np.asarray(event_lengths).astype(np.int64)
    in_maps, (qb, slots) = make_in_maps(x, lens, Wq, bq, Wk, bk, Wv, bv)
    if qb not in _NC_CACHE:
        _NC_CACHE[qb] = build_attention_nc(qb)
    nc = _NC_CACHE[qb]
    res = run_bass_kernel_spmd(nc, in_maps, core_ids=list(range(8)))
    # host assembly: scatter core blocks, then fill padded-query rows with
    # the exact mean of v over all N rows (softmax over an all-masked row).
    Wv32 = np.asarray(Wv, np.float32)
    bv32 = np.asarray(bv, np.float32)
    mean_v = (x.sum(axis=1) @ Wv32.T) / np.float32(N) + bv32  # [B, D]
    out = np.empty((B, N, D), np.float32)
    for c in range(8):
        co = np.asarray(res.results[c]["out"], np.float32)
        for j, slot in enumerate(slots[c]):
            if slot is None:
                continue
            b, blk = slot
            out[b, blk * FB : (blk + 1) * FB, :] = co[j * FB : (j + 1) * FB, :]
    for b in range(B):
        L = int(lens[b])
        if L < N:
            out[b, L:, :] = mean_v[b][None, :]
    return out
